# revision 26
# baseline (speedup 1.0000x reference)
"""Trainium2 Bass kernel for nn_Block_5583457485503 (mini transformer block).

Reference math (B=8192, T=32, C=128, H=4, D=32):
    q,k,v = per-head projections of x;  att = softmax(mask(q k^T / sqrt(D)))
    x = x + concat_h(att_h @ v_h);  x = x + relu(x@w1+b1)@w2 + b2

Sharding: data-parallel over batch across 8 cores (1024 seqs / core).
Weights replicated. Inside a core, tokens are processed in groups of
512 (16 seqs); within a group, 4 subtiles of 128 tokens (4 seqs each:
partition p = 32*b4 + s for quad-member b4, token-in-seq s).

Key trick: scores_h = x A_h x^T with A_h = wq_h wk_h^T * D^-0.5 computed
on host, so every PE contraction is 128-deep. An appended ones-column on
V yields the softmax denominator from the same matmuls that compute
att @ v.
"""

import os
import sys

import numpy as np

sys.path.insert(0, "/opt/trn_rl_repo")

NUM_EMB = 128
CONTEXT = 32
NUM_HEADS = 4
HEAD = 32
BATCH = 8192
N_CORES = 8
SEQ_PER_CORE = BATCH // N_CORES            # 1024
TOK_PER_CORE = SEQ_PER_CORE * CONTEXT      # 32768
GROUP_TOK = 512                            # tokens per group (16 seqs)
N_GROUPS = TOK_PER_CORE // GROUP_TOK       # 64


def _build_consts(wq, wk, wv, w1, b1, w2, b2):
    """Host-side constant prep (all fp32 numpy)."""
    f32 = np.float32
    # A_h = wq_h @ wk_h^T * D^-0.5   [H, C, C]
    a_all = np.einsum("hcd,hed->hce", wq, wk).astype(f32) * np.float32(HEAD ** -0.5)
    # Wv concat over heads: [C, H*D]
    wv_c = np.ascontiguousarray(wv.transpose(1, 0, 2).reshape(NUM_EMB, NUM_EMB)).astype(f32)
    w1_c = np.ascontiguousarray(w1).astype(f32)                       # [128, 512]
    w2_c = np.ascontiguousarray(w2.reshape(4, 128, NUM_EMB)).astype(f32)  # [j, p, c]
    # additive causal mask bias, tiled for [(b4,s), (h,t)] layout
    s_idx = np.arange(CONTEXT)[:, None]
    t_idx = np.arange(CONTEXT)[None, :]
    m0 = np.where(s_idx <= t_idx, 0.0, -1e9).astype(f32)   # [s, t]
    maskb = np.tile(m0, (4, NUM_HEADS))                     # [128, 128]
    ident = np.eye(128, dtype=f32)
    return dict(
        a_all=a_all, wv_c=wv_c, w1_c=w1_c, w2_c=w2_c,
        maskb=np.ascontiguousarray(maskb), ident=ident,
        b1_c=np.ascontiguousarray(b1).astype(f32).reshape(1, 4 * NUM_EMB),
        b2_c=np.ascontiguousarray(b2).astype(f32).reshape(1, NUM_EMB),
    )


def emit(ctx, tc, aps, n_groups):
    """Emit the per-core program.

    aps: dict of bass.AP handles keyed: xin, out, a_all, wv_c, w1_c, w2_c,
         maskb, ident, b1_c, b2_c
    """
    import concourse.bass as bass
    from concourse import mybir

    nc = tc.nc
    f32 = mybir.dt.float32
    AF = mybir.ActivationFunctionType

    use_b1 = aps.get("use_b1", False)
    use_b2 = aps.get("use_b2", False)

    consts = ctx.enter_context(tc.tile_pool(name="consts", bufs=1))
    sb = ctx.enter_context(tc.tile_pool(name="sb", bufs=2))
    # PSUM budget is 8 banks total: xt(1) p(2) sv(1) sc(1) op(1) h(1) y(1)
    ps = ctx.enter_context(tc.tile_pool(name="ps", bufs=1, space="PSUM"))
    ps2 = ctx.enter_context(tc.tile_pool(name="ps2", bufs=2, space="PSUM"))

    # ---- load constants to SBUF ----
    A_sb = consts.tile([128, NUM_HEADS, 128], f32)            # [c, h, c']
    nc.sync.dma_start(out=A_sb, in_=aps["a_all"].rearrange("h c e -> c h e"))
    Wv_sb = consts.tile([128, 128], f32)                       # [c, (h d)]
    nc.sync.dma_start(out=Wv_sb, in_=aps["wv_c"])
    W1_sb = consts.tile([128, 512], f32)                       # [c, 4c]
    nc.sync.dma_start(out=W1_sb, in_=aps["w1_c"])
    W2_sb = consts.tile([128, 4, 128], f32)                    # [p, j, c]
    nc.sync.dma_start(out=W2_sb, in_=aps["w2_c"].rearrange("j p c -> p j c"))
    Mk_sb = consts.tile([128, 128], f32)                       # [(b4 s), (h t)]
    nc.sync.dma_start(out=Mk_sb, in_=aps["maskb"])
    I_sb = consts.tile([128, 128], f32)
    nc.sync.dma_start(out=I_sb, in_=aps["ident"])
    if use_b1 or use_b2:
        ones_sb = consts.tile([1, 512], f32)
        nc.vector.memset(ones_sb, 1.0)
    if use_b1:
        B1_sb = consts.tile([1, 512], f32)
        nc.sync.dma_start(out=B1_sb, in_=aps["b1_c"])
    if use_b2:
        B2_sb = consts.tile([1, 128], f32)
        nc.sync.dma_start(out=B2_sb, in_=aps["b2_c"])

    xin, out = aps["xin"], aps["out"]
    repeats = aps.get("repeats", 1)

    def per_group(g):
        rows = slice(g * GROUP_TOK, (g + 1) * GROUP_TOK)
        # ---- load x group: [128 part, k-subtile, c] ----
        Xn = sb.tile([128, 4, 128], f32, tag="Xn")
        nc.sync.dma_start(out=Xn, in_=xin[rows, :].rearrange("(k p) c -> p k c", p=128))

        # ---- X^T via PE (regular matmul vs identity: out = Xn.T @ I).
        # Regular mode, not is_transpose: transpose-mode LDW only carries
        # one sync-wait and walrus rejects the 2-wait case. ----
        XTp = ps.tile([128, 4, 128], f32, tag="xt")
        for k in range(4):
            nc.tensor.matmul(XTp[:, k, :], lhsT=Xn[:, k, :], rhs=I_sb,
                             start=True, stop=True, skip_group_check=True)
        XT = sb.tile([128, 4, 128], f32, tag="XT")             # [c, k, tok]
        nc.scalar.copy(XT, XTp)

        # ---- P^T_h = A_h^T @ X^T : [c', (k tok)] per head ----
        PT = sb.tile([128, NUM_HEADS, 4, 128], f32, tag="PT")  # [c', h, k, tok]
        XTflat = XT.rearrange("c k t -> c (k t)")
        for h in range(NUM_HEADS):
            Pp = ps2.tile([128, 512], f32, tag="p")
            nc.tensor.matmul(Pp, lhsT=A_sb[:, h, :], rhs=XTflat, start=True, stop=True)
            dst = PT[:, h, :, :].rearrange("c k t -> c (k t)")
            if h % 2 == 0:
                nc.scalar.copy(dst, Pp)
            else:
                nc.vector.tensor_copy(dst, Pp)

        # ---- V = X @ Wv (+ones col): V33 [(b4 s), k, h, d|1] ----
        Vp = ps.tile([128, 4, 128], f32, tag="sv")
        for k in range(4):
            nc.tensor.matmul(Vp[:, k, :], lhsT=XT[:, k, :], rhs=Wv_sb,
                             start=True, stop=True, skip_group_check=True)
        V33 = sb.tile([128, 4, NUM_HEADS, HEAD + 1], f32, tag="V33")
        nc.vector.memset(V33[:, :, :, HEAD:HEAD + 1], 1.0)
        nc.vector.tensor_copy(V33[:, :, :, 0:HEAD],
                              Vp.rearrange("p k (h d) -> p k h d", h=NUM_HEADS))

        # ---- scores: Sp[(b4 s), k, (h t)] = x_b4 . P_h[t] ----
        Sp = ps.tile([128, 4, 128], f32, tag="sc")
        for k in range(4):
            for b4 in range(4):
                nc.tensor.matmul(
                    Sp[32 * b4:32 * b4 + 32, k, :],
                    lhsT=XT[:, k, 32 * b4:32 * b4 + 32],
                    rhs=PT[:, :, k, 32 * b4:32 * b4 + 32],
                    start=True, stop=True, skip_group_check=True,
                    tile_position=(0, 32 * b4))

        # ---- mask + exp ----
        Sm = sb.tile([128, 4, 128], f32, tag="Sm")
        nc.vector.tensor_add(Sm, Sp, Mk_sb.unsqueeze(1).to_broadcast([128, 4, 128]))
        E = sb.tile([128, 4, 128], f32, tag="E")
        nc.scalar.activation(E.rearrange("p k t -> p (k t)"),
                             Sm.rearrange("p k t -> p (k t)"), AF.Exp)

        # ---- phase 2: O'[t, h, d|denom] = sum_s att * V33 (diag tiles) ----
        Zn = sb.tile([128, 4, 128], f32, tag="Zn")
        for kk in range(2):  # two psum banks, 2 subtiles each
            # full-bank tile viewed as [k2, h, 64] so [*,33] spans never
            # cross the 2KB bank boundary
            Opb = ps.tile([128, 512], f32, tag="op")
            Op = Opb.rearrange("p (a h d) -> p a h d", a=2, h=NUM_HEADS)
            for k2 in range(2):
                k = 2 * kk + k2
                for b4 in range(4):
                    pr = slice(32 * b4, 32 * b4 + 32)
                    for h in range(NUM_HEADS):
                        nc.tensor.matmul(
                            Op[pr, k2, h, 0:HEAD + 1],
                            lhsT=E[pr, k, 32 * h:32 * h + 32],
                            rhs=V33[pr, k, h, :],
                            start=True, stop=True,
                            skip_group_check=True,
                            tile_position=(32 * b4, 32 * b4))
            for k2 in range(2):
                k = 2 * kk + k2
                R = sb.tile([128, NUM_HEADS], f32, tag="R")
                nc.vector.reciprocal(R, Op[:, k2, :, HEAD:HEAD + 1].squeeze(2))
                tmp = sb.tile([128, NUM_HEADS, HEAD], f32, tag="tmp")
                nc.vector.tensor_mul(
                    tmp, Op[:, k2, :, 0:HEAD],
                    R.unsqueeze(2).to_broadcast([128, NUM_HEADS, HEAD]))
                nc.vector.tensor_add(Zn[:, k, :],
                                     tmp.rearrange("p h d -> p (h d)"), Xn[:, k, :])

        # ---- Z^T (regular matmul vs identity, see X^T note) ----
        ZTp = ps.tile([128, 4, 128], f32, tag="xt")
        for k in range(4):
            nc.tensor.matmul(ZTp[:, k, :], lhsT=Zn[:, k, :], rhs=I_sb,
                             start=True, stop=True, skip_group_check=True)
        ZT = sb.tile([128, 4, 128], f32, tag="ZT")
        nc.scalar.copy(ZT, ZTp)
        ZTflat = ZT.rearrange("c k t -> c (k t)")

        # ---- FFN1: H^T chunks [4c_j, (k tok)], relu ----
        Hs = sb.tile([128, 4, 512], f32, tag="Hs")
        for j in range(4):
            Hp = ps.tile([128, 512], f32, tag="h")
            if use_b1:
                nc.tensor.matmul(Hp, lhsT=B1_sb[:, 128 * j:128 * j + 128],
                                 rhs=ones_sb, start=True, stop=False,
                                 skip_group_check=True)
            nc.tensor.matmul(Hp, lhsT=W1_sb[:, 128 * j:128 * j + 128], rhs=ZTflat,
                             start=not use_b1, stop=True, skip_group_check=True)
            nc.scalar.activation(Hs[:, j, :], Hp, AF.Relu)

        # ---- FFN2 + residual ----
        Yo = sb.tile([128, 4, 128], f32, tag="Yo")
        Yp = ps.tile([128, 4, 128], f32, tag="y")
        for k in range(4):
            if use_b2:
                nc.tensor.matmul(Yp[:, k, :], lhsT=ones_sb[:, 0:128], rhs=B2_sb,
                                 start=True, stop=False, skip_group_check=True)
            for j in range(4):
                nc.tensor.matmul(Yp[:, k, :], lhsT=Hs[:, j, 128 * k:128 * k + 128],
                                 rhs=W2_sb[:, j, :],
                                 start=(j == 0 and not use_b2), stop=(j == 3),
                                 skip_group_check=True)
            nc.vector.tensor_add(Yo[:, k, :], Yp[:, k, :], Zn[:, k, :])

        nc.sync.dma_start(out=out[rows, :].rearrange("(k p) c -> p k c", p=128),
                          in_=Yo)

    if repeats > 1:
        with tc.For_i(0, repeats, 1):
            for g in range(n_groups):
                per_group(g)
    else:
        for g in range(n_groups):
            per_group(g)


def build_program(n_groups, use_b1=False, use_b2=False, repeats=1):
    """Build Bass program; returns (nc, input_names)."""
    from contextlib import ExitStack

    import concourse.bass as bass
    import concourse.tile as tile
    from concourse import bacc, mybir

    f32 = mybir.dt.float32
    nc = bacc.Bacc(trn_type="TRN2")
    tok = n_groups * GROUP_TOK
    aps = {
        "xin": nc.dram_tensor("xin", [tok, 128], f32, kind="ExternalInput")[:, :],
        "a_all": nc.dram_tensor("a_all", [NUM_HEADS, 128, 128], f32, kind="ExternalInput")[:, :, :],
        "wv_c": nc.dram_tensor("wv_c", [128, 128], f32, kind="ExternalInput")[:, :],
        "w1_c": nc.dram_tensor("w1_c", [128, 512], f32, kind="ExternalInput")[:, :],
        "w2_c": nc.dram_tensor("w2_c", [4, 128, 128], f32, kind="ExternalInput")[:, :, :],
        "maskb": nc.dram_tensor("maskb", [128, 128], f32, kind="ExternalInput")[:, :],
        "ident": nc.dram_tensor("ident", [128, 128], f32, kind="ExternalInput")[:, :],
        "out": nc.dram_tensor("out", [tok, 128], f32, kind="ExternalOutput")[:, :],
        "use_b1": use_b1, "use_b2": use_b2, "repeats": repeats,
    }
    if use_b1:
        aps["b1_c"] = nc.dram_tensor("b1_c", [1, 512], f32, kind="ExternalInput")[:, :]
    if use_b2:
        aps["b2_c"] = nc.dram_tensor("b2_c", [1, 128], f32, kind="ExternalInput")[:, :]

    with ExitStack() as ctx:
        tc = ctx.enter_context(tile.TileContext(nc))
        emit(ctx, tc, aps, n_groups)
    nc.compile()
    return nc


_LAST_RESULTS = None  # BassKernelResults from the most recent kernel() call


def kernel(x, wq, wk, wv, w1, b1, w2, b2):
    """Full-input entry point: shards x over 8 cores, runs on HW, gathers."""
    global _LAST_RESULTS
    from concourse.bass_utils import run_bass_kernel_spmd

    in_maps, use_b1, use_b2 = _make_in_maps(x, wq, wk, wv, w1, b1, w2, b2)
    nc = build_program(N_GROUPS, use_b1, use_b2)
    res = run_bass_kernel_spmd(nc, in_maps, list(range(N_CORES)))
    _LAST_RESULTS = res
    out = np.concatenate([res.results[i]["out"] for i in range(N_CORES)], axis=0)
    return out.reshape(BATCH, CONTEXT, NUM_EMB).astype(np.float32)


def _make_in_maps(x, wq, wk, wv, w1, b1, w2, b2):
    consts = _build_consts(np.asarray(wq, np.float32), np.asarray(wk, np.float32),
                           np.asarray(wv, np.float32), np.asarray(w1, np.float32),
                           np.asarray(b1, np.float32), np.asarray(w2, np.float32),
                           np.asarray(b2, np.float32))
    use_b1 = bool(np.any(consts["b1_c"]))
    use_b2 = bool(np.any(consts["b2_c"]))
    const_map = {
        "a_all": consts["a_all"], "wv_c": consts["wv_c"], "w1_c": consts["w1_c"],
        "w2_c": consts["w2_c"], "maskb": consts["maskb"], "ident": consts["ident"],
    }
    if use_b1:
        const_map["b1_c"] = consts["b1_c"]
    if use_b2:
        const_map["b2_c"] = consts["b2_c"]
    shards = np.asarray(x, np.float32).reshape(N_CORES, TOK_PER_CORE, NUM_EMB)
    in_maps = [dict(xin=np.ascontiguousarray(shards[i]), **const_map)
               for i in range(N_CORES)]
    return in_maps, use_b1, use_b2


def bench_exec_time(np_inputs, r_hi=9, reps=3):
    """Device-time estimate via repeat-loop slope: (t(r_hi) - t(1)) / (r_hi-1).

    Transfer/dispatch costs are identical for both programs and cancel.
    Returns ns per single pass over the full workload.
    """
    import time

    from concourse.bass_utils import run_bass_kernel_spmd

    in_maps, use_b1, use_b2 = _make_in_maps(**np_inputs)

    def timed(repeats):
        nc = build_program(N_GROUPS, use_b1, use_b2, repeats=repeats)
        best = None
        for _ in range(reps):
            t0 = time.perf_counter()
            run_bass_kernel_spmd(nc, in_maps, list(range(N_CORES)))
            dt = time.perf_counter() - t0
            best = dt if best is None else min(best, dt)
        return best

    t1 = timed(1)
    th = timed(r_hi)
    ns = (th - t1) / (r_hi - 1) * 1e9
    print(f"bench: t(1)={t1*1e3:.1f} ms  t({r_hi})={th*1e3:.1f} ms  -> {ns:.0f} ns/pass")
    return ns
